# revision 1
# baseline (speedup 1.0000x reference)
"""Causal self-attention block (B=32, T=512, C=768, H=12) on 8 Trainium2 cores.

Strategy: data-parallel over batch (4 batches per core). All matmuls in bf16
with fp32 PSUM accumulation. The dataflow is arranged so no on-chip
transposes are needed:

  host:   xT[b] = x[b].T  (bf16, [C, T])
  qkT  [2C, T] = W_qk.T @ x.T      (lhsT = natural W_qk slices, rhs = xT)
  v    [T, C]  = x @ W_v           (lhsT = xT slices,           rhs = W_v)
  sT_h [Tk, Tq] = k_h q_h.T        (lhsT = kT_h slice,  rhs = qT_h slice, K=D)
  pT_h = exp(sT/sqrt(D)) * causal  (scalar engine; no max-sub needed: |s/8|<~2)
  o_h  [D+1, Tq] = [v_h | 1].T @ pT_h   (row D = softmax denominator l)
  oT_h = o_h[:D] * (1/l)           (DVE recip + gpsimd partition_broadcast)
  y    [T, C]  = o @ W_proj        (lhsT = oT slices, rhs = natural W_proj)

Causality is exploited at tile granularity: for k-tile i only q >= 128*i is
computed; the diagonal 128x128 chunk is masked with a 0/1 bf16 mask.
"""
import sys

sys.path.insert(0, "/opt/trn_rl_repo")

import numpy as np
import ml_dtypes

import concourse.bass as bass
import concourse.tile as tile
import concourse.mybir as mybir
from concourse import bacc, library_config
from concourse.bass_utils import run_bass_kernel_spmd

F32 = mybir.dt.float32
F32R = mybir.dt.float32r
BF16 = mybir.dt.bfloat16
AF = mybir.ActivationFunctionType
BF16NP = ml_dtypes.bfloat16

B, T, C = 32, 512, 768
H = 12
D = C // H  # 64
NCORES = 8
NB = B // NCORES  # batches per core
KT = C // 128  # 6 contraction tiles
MQK = (2 * C) // 128  # 12 output tiles for q|k features
TT = T // 128  # 4 token tiles
SCALE = 1.0 / np.sqrt(D)


DEFAULT_CFG = dict(
    xt=2, qkt=2, va=2, ot=2, pt=6, small=6, mm=2, st=2, o=2, y=2,
    ou=2, norm_pool=0, pairs=0, tail4=1, g2first=0, g4defer=1, div=0, split2=0, ysb=4, dma2=0, ysplit=1,
)


def build_bass(cfg=None):
    cfg = {**DEFAULT_CFG, **(cfg or {})}
    nc = bacc.Bacc()

    xT_d = nc.dram_tensor("xT", [NB, C, T], BF16, kind="ExternalInput")
    wqk_d = nc.dram_tensor("wqk", [C, 2 * C], BF16, kind="ExternalInput")
    wv_d = nc.dram_tensor("wv", [C, C], BF16, kind="ExternalInput")
    wp_d = nc.dram_tensor("wp", [C, C], BF16, kind="ExternalInput")
    bqk_d = nc.dram_tensor("bqk", [128, MQK], F32, kind="ExternalInput")
    bv_d = nc.dram_tensor("bv", [128, C], F32, kind="ExternalInput")
    bp_d = nc.dram_tensor("bp", [128, C], F32, kind="ExternalInput")
    mask_d = nc.dram_tensor("mask", [128, 128], BF16, kind="ExternalInput")
    y_d = nc.dram_tensor("y", [NB, T, C], F32, kind="ExternalOutput")

    with tile.TileContext(nc) as tc:
        with (
            tc.tile_pool(name="consts", bufs=1) as consts,
            tc.tile_pool(name="xt", bufs=cfg["xt"]) as xt_pool,
            tc.tile_pool(name="qkt", bufs=cfg["qkt"]) as qkt_pool,
            tc.tile_pool(name="va", bufs=cfg["va"]) as va_pool,
            tc.tile_pool(name="ot", bufs=cfg["ot"]) as ot_pool,
            tc.tile_pool(name="pt", bufs=cfg["pt"]) as pt_pool,
            tc.tile_pool(name="small", bufs=cfg["small"]) as small_pool,
            tc.tile_pool(name="ysb", bufs=cfg["ysb"]) as y_pool,
            tc.tile_pool(name="psmm", bufs=cfg["mm"], space="PSUM") as ps_mm,
            tc.tile_pool(name="psst", bufs=cfg["st"], space="PSUM") as ps_st,
            tc.tile_pool(name="pso", bufs=cfg["o"], space="PSUM") as ps_o,
            tc.tile_pool(name="psy", bufs=max(cfg["y"], 1), space="PSUM") as ps_y,
        ):
            # ---- constants (issue order = need order: XT[0], Wqk, then rest) ----
            XT0 = xt_pool.tile([128, KT, T], BF16, tag="xt")
            xt0_r = xT_d[0].rearrange("(k p) t -> p k t", p=128)
            Wv = consts.tile([128, KT, C], BF16)
            wv_r = wv_d.rearrange("(k p) n -> p k n", p=128)
            Wqk = consts.tile([128, KT, 2 * C], BF16)
            wqk_r = wqk_d.rearrange("(k p) n -> p k n", p=128)
            if cfg["g2first"]:
                for k in range(KT):
                    nc.sync.dma_start(XT0[:, k, :], xt0_r[:, k, :])
                    nc.sync.dma_start(Wv[:, k, :], wv_r[:, k, :])
                Bv = consts.tile([128, C], F32)
                nc.sync.dma_start(Bv, bv_d[:])
                for k in range(KT):
                    nc.sync.dma_start(Wqk[:, k, :], wqk_r[:, k, :])
                Bqk = consts.tile([128, MQK], F32)
                nc.sync.dma_start(Bqk, bqk_d[:])
                Mask = consts.tile([128, 128], BF16)
                nc.sync.dma_start(Mask, mask_d[:])
            else:
                weng = nc.gpsimd if cfg["dma2"] else nc.sync
                for k in range(KT):
                    nc.sync.dma_start(XT0[:, k, :], xt0_r[:, k, :])
                    weng.dma_start(Wqk[:, k, :], wqk_r[:, k, :])
                Bqk = consts.tile([128, MQK], F32)
                nc.sync.dma_start(Bqk, bqk_d[:])
                nc.sync.dma_start(Wv, wv_r)
                Mask = consts.tile([128, 128], BF16)
                nc.sync.dma_start(Mask, mask_d[:])
                Bv = consts.tile([128, C], F32)
                nc.sync.dma_start(Bv, bv_d[:])
            Wp = consts.tile([128, KT, C], BF16)
            nc.sync.dma_start(Wp, wp_d.rearrange("(k p) n -> p k n", p=128))
            Bp = consts.tile([128, C], F32)
            nc.sync.dma_start(Bp, bp_d[:])
            nc.gpsimd.load_library(library_config.attn)
            pending_g4 = []

            for b in range(NB):
                # ---- load xT for this batch ----
                if b == 0:
                    XT = XT0
                else:
                    XT = xt_pool.tile([128, KT, T], BF16, tag="xt")
                    nc.sync.dma_start(XT, xT_d[b].rearrange("(k p) t -> p k t", p=128))

                # ---- GEMM1: qkT [2C, T], feature-major ----
                # emit m-tiles in (q-tile, k-tile) pairs so head h unblocks
                # after 2 m-tiles instead of after the whole q half
                QKT = qkt_pool.tile([128, MQK, T], BF16)

                def gemm1():
                    m_order = [
                        m for qt_ in range(MQK // 2) for m in (qt_, MQK // 2 + qt_)
                    ]
                    for m in m_order:
                        qk_ps = ps_mm.tile([128, T], F32, tag="mm")
                        for k in range(KT):
                            nc.tensor.matmul(
                                qk_ps,
                                Wqk[:, k, 128 * m : 128 * (m + 1)],
                                XT[:, k, :],
                                start=(k == 0),
                                stop=(k == KT - 1),
                            )
                        nc.scalar.activation(
                            QKT[:, m, :], qk_ps, AF.Identity, bias=Bqk[:, m : m + 1]
                        )

                # ---- GEMM2: v_aug [T, H, D+1], token-major with ones column ----
                VA = va_pool.tile([128, TT, H, D + 1], BF16)

                def gemm2():
                    nc.vector.memset(VA[:, :, :, D : D + 1], 1.0)
                    for t in range(TT):
                        for n0, nw in ((0, 512), (512, 256)):
                            v_ps = ps_mm.tile([128, T], F32, tag="mm")
                            for k in range(KT):
                                nc.tensor.matmul(
                                    v_ps[:, :nw],
                                    XT[:, k, 128 * t : 128 * (t + 1)],
                                    Wv[:, k, n0 : n0 + nw],
                                    start=(k == 0),
                                    stop=(k == KT - 1),
                                )
                            nc.vector.tensor_tensor(
                                VA[:, t, n0 // D : (n0 + nw) // D, 0:D],
                                v_ps[:, :nw].rearrange("p (h d) -> p h d", d=D),
                                Bv[:, n0 : n0 + nw].rearrange("p (h d) -> p h d", d=D),
                                mybir.AluOpType.add,
                            )

                # batch 0: the v weights (2MB) arrive before Wqk (3.5MB), so
                # GEMM2 first gets the PE going ~2us earlier
                if b == 0 and cfg["g2first"]:
                    gemm2()
                    gemm1()
                else:
                    gemm1()
                    gemm2()

                # ---- attention per head ----
                OT = ot_pool.tile([128, KT, T], BF16)

                def head_slices(h):
                    qt = h // 2
                    qr = D * (h % 2)
                    return (
                        QKT[qr : qr + D, qt, :],
                        QKT[qr : qr + D, MQK // 2 + qt, :],
                    )

                def st_exp_av(h, i, o_ps):
                    qT_h, kT_h = head_slices(h)
                    n = T - 128 * i
                    st_ps = ps_st.tile([128, T], F32, tag="st")
                    nc.tensor.matmul(
                        st_ps[:, :n],
                        kT_h[:, 128 * i : 128 * (i + 1)],
                        qT_h[:, 128 * i : T],
                        start=True,
                        stop=True,
                    )
                    PT = pt_pool.tile([128, T], BF16)
                    if cfg["split2"] and n >= 256:
                        chunks = [(0, n // 256 * 128), (n // 256 * 128, n)]
                    else:
                        chunks = [(0, n)]
                    for c0, c1 in chunks:
                        nc.scalar.activation(
                            PT[:, c0:c1], st_ps[:, c0:c1], AF.Exp, scale=SCALE
                        )
                        if c0 == 0:
                            nc.vector.tensor_tensor(
                                PT[:, 0:128], PT[:, 0:128], Mask, mybir.AluOpType.mult
                            )
                        nc.tensor.matmul(
                            o_ps[0 : D + 1, 128 * i + c0 : 128 * i + c1],
                            VA[:, i, h, :],
                            PT[:, c0:c1],
                            start=(i == 0 and c0 == 0),
                            stop=(i == TT - 1 and c1 == n),
                        )

                def normalize(h, o_ps):
                    # normalize: oT_h = o[:D] / l, l = o row D
                    qt = h // 2
                    qr = D * (h % 2)
                    rinv = small_pool.tile([1, T], F32, tag="rinv")
                    if cfg["div"] == 0:
                        nc.vector.reciprocal(rinv, o_ps[D : D + 1, :])
                    elif cfg["div"] == 1:
                        nc.scalar.copy(rinv, o_ps[D : D + 1, :])
                    elif cfg["div"] == 2:
                        nc.vector.tensor_copy(rinv, o_ps[D : D + 1, :])
                    else:
                        nc.any.tensor_copy(rinv, o_ps[D : D + 1, :])
                    norm_op = (
                        mybir.AluOpType.mult if cfg["div"] == 0 else mybir.AluOpType.divide
                    )
                    rb = small_pool.tile([D, T], F32, tag="rb_sb")
                    nc.gpsimd.partition_broadcast(rb, rinv[:])
                    if cfg["ou"] == 0:
                        nc.vector.tensor_tensor(
                            OT[qr : qr + D, qt, :],
                            o_ps[0:D, :],
                            rb,
                            norm_op,
                        )
                    else:
                        oU = small_pool.tile([D, T], F32, tag="ou_sb")
                        if cfg["ou"] == 1:
                            nc.scalar.copy(oU, o_ps[0:D, :])
                        elif cfg["ou"] == 4:
                            nc.any.tensor_copy(oU, o_ps[0:D, :])
                        elif cfg["ou"] == 5 and h % 2 == 1:
                            nc.scalar.copy(oU, o_ps[0:D, :])
                        else:
                            nc.vector.tensor_copy(oU, o_ps[0:D, :])
                        eng = nc.gpsimd if cfg["norm_pool"] else nc.vector
                        eng.tensor_tensor(
                            OT[qr : qr + D, qt, :],
                            oU,
                            rb,
                            norm_op,
                        )

                if cfg["pairs"]:
                    # paired emission: the two heads of a QKT tile alternate, so
                    # their K=64 ST matmuls sit adjacently at row groups 0/64
                    # (concurrent on HW via tile_position row packing)
                    for pair in range(H // 2):
                        hA, hB = 2 * pair, 2 * pair + 1
                        oA = ps_o.tile([128, T], F32, tag="o")
                        oB = ps_o.tile([128, T], F32, tag="o")
                        for i in range(TT):
                            st_exp_av(hA, i, oA)
                            st_exp_av(hB, i, oB)
                        normalize(hA, oA)
                        normalize(hB, oB)
                else:
                    for h in range(H):
                        o_ps = ps_o.tile([128, T], F32, tag="o")
                        for i in range(TT):
                            st_exp_av(h, i, o_ps)
                        normalize(h, o_ps)

                # ---- GEMM4: y = o @ W_proj + b ----
                def gemm4(b=b, OT=OT):
                    last = b == NB - 1
                    for t in range(TT):
                        y_sb = y_pool.tile([128, C], F32, tag="ysb")
                        for ci, (n0, nw) in enumerate(((0, 512), (512, 256))):
                            # last batch: no next-batch GEMM1 needs the mm slots,
                            # so alternate pools for 4 accumulation groups in flight
                            if cfg["y"] == 0 or (
                                cfg["tail4"] and last and (2 * t + ci) % 2 == 1
                            ):
                                y_ps = ps_mm.tile([128, T], F32, tag="mm")
                            else:
                                y_ps = ps_y.tile([128, T], F32, tag="y")
                            for k in range(KT):
                                nc.tensor.matmul(
                                    y_ps[:, :nw],
                                    OT[:, k, 128 * t : 128 * (t + 1)],
                                    Wp[:, k, n0 : n0 + nw],
                                    start=(k == 0),
                                    stop=(k == KT - 1),
                                )
                            nc.vector.tensor_tensor(
                                y_sb[:, n0 : n0 + nw],
                                y_ps[:, :nw],
                                Bp[:, n0 : n0 + nw],
                                mybir.AluOpType.add,
                            )
                        if cfg["ysplit"] and last:
                            for n0, nw in ((0, 512), (512, 256)):
                                nc.sync.dma_start(
                                    y_d[b, 128 * t : 128 * (t + 1), n0 : n0 + nw],
                                    y_sb[:, n0 : n0 + nw],
                                )
                        else:
                            nc.sync.dma_start(y_d[b, 128 * t : 128 * (t + 1), :], y_sb)

                if cfg["g4defer"]:
                    pending_g4.append(gemm4)
                    if b >= 1:
                        pending_g4.pop(0)()
                else:
                    gemm4()

            if cfg["g4defer"]:
                for fn in pending_g4:
                    fn()

    return nc


_NC_CACHE = None


def _get_nc():
    global _NC_CACHE
    if _NC_CACHE is None:
        nc = build_bass()
        nc.finalize()
        _NC_CACHE = nc
    return _NC_CACHE


def make_in_maps(x, w_qkv, b_qkv, b_proj, w_proj):
    x = np.asarray(x, np.float32)
    w_qkv = np.asarray(w_qkv, np.float32)
    b_qkv = np.asarray(b_qkv, np.float32)
    w_proj = np.asarray(w_proj, np.float32)
    b_proj = np.asarray(b_proj, np.float32)
    wqk = np.ascontiguousarray(w_qkv[:, : 2 * C]).astype(BF16NP)
    wv = np.ascontiguousarray(w_qkv[:, 2 * C :]).astype(BF16NP)
    wp = np.asarray(w_proj).astype(BF16NP)
    bqk = np.ascontiguousarray(
        np.asarray(b_qkv[: 2 * C], np.float32).reshape(MQK, 128).T
    )
    bv = np.broadcast_to(np.asarray(b_qkv[2 * C :], np.float32), (128, C)).copy()
    bp = np.broadcast_to(np.asarray(b_proj, np.float32), (128, C)).copy()
    kk, qq = np.meshgrid(np.arange(128), np.arange(128), indexing="ij")
    mask = (kk <= qq).astype(BF16NP)

    in_maps = []
    for c in range(NCORES):
        xc = np.asarray(x[c * NB : (c + 1) * NB], np.float32)
        xT = np.ascontiguousarray(xc.transpose(0, 2, 1)).astype(BF16NP)
        in_maps.append(
            {
                "xT": xT,
                "wqk": wqk,
                "wv": wv,
                "wp": wp,
                "bqk": bqk,
                "bv": bv,
                "bp": bp,
                "mask": mask,
            }
        )
    return in_maps


def kernel(x, w_qkv, b_qkv, w_proj, b_proj, _trace=False, _tmpdir=None):
    x = np.asarray(x)
    in_maps = make_in_maps(x, w_qkv, b_qkv, b_proj, w_proj)
    nc = _get_nc()
    res = run_bass_kernel_spmd(
        nc, in_maps, list(range(NCORES)), trace=_trace, tmpdir=_tmpdir
    )
    out = np.concatenate([np.asarray(r["y"], np.float32) for r in res.results], axis=0)
    if _trace:
        kernel.last_exec_time_ns = res.exec_time_ns
        kernel.last_results = res
    return out.reshape(B, T, C)


if __name__ == "__main__":
    rng = np.random.default_rng(0)
    x = rng.standard_normal((B, T, C), dtype=np.float32)
    w_qkv = (rng.standard_normal((C, 3 * C), dtype=np.float32) * 0.02).astype(np.float32)
    b_qkv = np.zeros((3 * C,), np.float32)
    w_proj = (rng.standard_normal((C, C), dtype=np.float32) * 0.02).astype(np.float32)
    b_proj = np.zeros((C,), np.float32)
    y = kernel(x, w_qkv=w_qkv, b_qkv=b_qkv, w_proj=w_proj, b_proj=b_proj)
    print(y.shape, y.dtype)



# revision 28
# speedup vs baseline: 1.3450x; 1.3450x over previous
"""Causal self-attention block (B=32, T=512, C=768, H=12) on 8 Trainium2 cores.

Strategy: data-parallel over batch (4 batches per core). All matmuls in bf16
with fp32 PSUM accumulation. The dataflow is arranged so no on-chip
transposes are needed:

  host:   xT[b] = x[b].T  (bf16, [C, T])
  qkT  [2C, T] = W_qk.T @ x.T      (lhsT = natural W_qk slices, rhs = xT)
  v    [T, C]  = x @ W_v           (lhsT = xT slices,           rhs = W_v)
  sT_h [Tk, Tq] = k_h q_h.T        (lhsT = kT_h slice,  rhs = qT_h slice, K=D)
  pT_h = exp(sT/sqrt(D)) * causal  (scalar engine; no max-sub needed: |s/8|<~2)
  o_h  [D+1, Tq] = [v_h | 1].T @ pT_h   (row D = softmax denominator l)
  oT_h = o_h[:D] * (1/l)           (DVE recip + gpsimd partition_broadcast)
  y    [T, C]  = o @ W_proj        (lhsT = oT slices, rhs = natural W_proj)

Causality is exploited at tile granularity: for k-tile i only q >= 128*i is
computed; the diagonal 128x128 chunk is masked with a 0/1 bf16 mask.
"""
import sys

sys.path.insert(0, "/opt/trn_rl_repo")

import numpy as np
import ml_dtypes

import concourse.bass as bass
import concourse.tile as tile
import concourse.mybir as mybir
from concourse import bacc, library_config
from concourse.bass_utils import run_bass_kernel_spmd

F32 = mybir.dt.float32
F32R = mybir.dt.float32r
BF16 = mybir.dt.bfloat16
F8 = mybir.dt.float8e4
AF = mybir.ActivationFunctionType
BF16NP = ml_dtypes.bfloat16
F8NP = ml_dtypes.float8_e4m3
DR = mybir.MatmulPerfMode.DoubleRow
# fp8 per-tensor power-of-2 scales for GEMM1 (folded back out in the Act
# epilogue, which computes ps*scale + bias)
SX8 = 8.0
SW8 = 512.0

B, T, C = 32, 512, 768
H = 12
D = C // H  # 64
NCORES = 8
NB = B // NCORES  # batches per core
KT = C // 128  # 6 contraction tiles
MQK = (2 * C) // 128  # 12 output tiles for q|k features
TT = T // 128  # 4 token tiles
SCALE = 1.0 / np.sqrt(D)


DEFAULT_CFG = dict(
    xt=2, qkt=2, va=2, ot=2, pt=6, small=6, mm=2, st=2, o=2, y=1,
    ou=2, norm_pool=0, pairs=1, tail4=1, g2first=0, g4defer=1, div=0, split2=0, ysb=4, dma2=0, ysplit=1,
    g1f8=1, xt8=2,
    # token-major AV + PE transpose + per-partition normalize
    tmav=1, tp=1, maskp=0, ybf16=1, actwarm=1,
    expmerge=1, g1epi_dve=1, wqk8chunk=1,
)


def build_bass(cfg=None):
    cfg = {**DEFAULT_CFG, **(cfg or {})}
    nc = bacc.Bacc()

    xT_d = nc.dram_tensor("xT", [NB, C, T], BF16, kind="ExternalInput")
    if cfg["g1f8"]:
        xT8_d = nc.dram_tensor("xT8", [NB, C, T], F8, kind="ExternalInput")
        wqk8_d = nc.dram_tensor("wqk8", [C, 2 * C], F8, kind="ExternalInput")
    else:
        wqk_d = nc.dram_tensor("wqk", [C, 2 * C], BF16, kind="ExternalInput")
    wv_d = nc.dram_tensor("wv", [C, C], BF16, kind="ExternalInput")
    wp_d = nc.dram_tensor("wp", [C, C], BF16, kind="ExternalInput")
    bqk_d = nc.dram_tensor("bqk", [128, MQK], F32, kind="ExternalInput")
    bv_d = nc.dram_tensor("bv", [128, C], F32, kind="ExternalInput")
    bp_d = nc.dram_tensor("bp", [128, C], F32, kind="ExternalInput")
    mask_d = nc.dram_tensor("mask", [128, 128], BF16, kind="ExternalInput")
    if cfg["tmav"]:
        ident_d = nc.dram_tensor("ident", [128, 128], BF16, kind="ExternalInput")
    y_d = nc.dram_tensor("y", [NB, T, C], BF16 if cfg["ybf16"] else F32,
                         kind="ExternalOutput")

    with tile.TileContext(nc) as tc:
        with (
            tc.tile_pool(name="consts", bufs=1) as consts,
            tc.tile_pool(name="xt", bufs=cfg["xt"]) as xt_pool,
            tc.tile_pool(name="xt8", bufs=cfg["xt8"]) as xt8_pool,
            tc.tile_pool(name="qkt", bufs=cfg["qkt"]) as qkt_pool,
            tc.tile_pool(name="va", bufs=cfg["va"]) as va_pool,
            tc.tile_pool(name="ot", bufs=cfg["ot"]) as ot_pool,
            tc.tile_pool(name="pt", bufs=cfg["pt"]) as pt_pool,
            tc.tile_pool(name="small", bufs=cfg["small"]) as small_pool,
            tc.tile_pool(name="ysb", bufs=cfg["ysb"]) as y_pool,
            tc.tile_pool(name="psmm", bufs=cfg["mm"], space="PSUM") as ps_mm,
            tc.tile_pool(name="psst", bufs=cfg["st"], space="PSUM") as ps_st,
            tc.tile_pool(name="pso", bufs=cfg["o"], space="PSUM") as ps_o,
            tc.tile_pool(name="pstp", bufs=max(cfg["tp"], 1), space="PSUM") as ps_tp,
            tc.tile_pool(name="psy", bufs=max(cfg["y"], 1), space="PSUM") as ps_y,
        ):
            if cfg["actwarm"]:
                # tiny activation up front so the act-table load (~1.3us) is
                # hoisted before it and overlaps the initial weight DMA
                warm = consts.tile([1, 2], F32)
                nc.vector.memset(warm, 0.0)
                nc.scalar.activation(warm, warm, AF.Exp)

            # ---- constants (issue order = need order: XT[0], Wqk, then rest) ----
            XT0 = xt_pool.tile([128, KT, T], BF16, tag="xt")
            xt0_r = xT_d[0].rearrange("(k p) t -> p k t", p=128)
            Wv = consts.tile([128, KT, C], BF16)
            wv_r = wv_d.rearrange("(k p) n -> p k n", p=128)
            if cfg["g1f8"]:
                # fp8 path: gemm1 operands first (XT8[0], then Wqk8 in
                # emission-column order so m-tiles unblock as chunks land)
                XT8_0 = xt8_pool.tile([128, KT, T], F8, tag="xt8")
                xt8_0r = xT8_d[0].rearrange("(k p) t -> p k t", p=128)
                nc.sync.dma_start(XT8_0, xt8_0r)
                Wqk8 = consts.tile([128, KT, 2 * C], F8)
                wqk8_r = wqk8_d.rearrange("(k p) n -> p k n", p=128)
                if cfg["wqk8chunk"]:
                    nc.sync.dma_start(Wqk8[:, :, :512], wqk8_r[:, :, :512])
                else:
                    nc.sync.dma_start(Wqk8, wqk8_r)
                Bqk = consts.tile([128, MQK], F32)
                nc.sync.dma_start(Bqk, bqk_d[:])
                Mask = consts.tile([128, 128], BF16)
                nc.sync.dma_start(Mask, mask_d[:])
                if cfg["wqk8chunk"]:
                    for j in range(1, 3):
                        nc.sync.dma_start(
                            Wqk8[:, :, 512 * j : 512 * (j + 1)],
                            wqk8_r[:, :, 512 * j : 512 * (j + 1)],
                        )
                nc.sync.dma_start(XT0, xt0_r)
                nc.sync.dma_start(Wv, wv_r)
                Bv = consts.tile([128, C], F32)
                nc.sync.dma_start(Bv, bv_d[:])
            elif cfg["g2first"]:
                Wqk = consts.tile([128, KT, 2 * C], BF16)
                wqk_r = wqk_d.rearrange("(k p) n -> p k n", p=128)
                for k in range(KT):
                    nc.sync.dma_start(XT0[:, k, :], xt0_r[:, k, :])
                    nc.sync.dma_start(Wv[:, k, :], wv_r[:, k, :])
                Bv = consts.tile([128, C], F32)
                nc.sync.dma_start(Bv, bv_d[:])
                for k in range(KT):
                    nc.sync.dma_start(Wqk[:, k, :], wqk_r[:, k, :])
                Bqk = consts.tile([128, MQK], F32)
                nc.sync.dma_start(Bqk, bqk_d[:])
                Mask = consts.tile([128, 128], BF16)
                nc.sync.dma_start(Mask, mask_d[:])
            else:
                Wqk = consts.tile([128, KT, 2 * C], BF16)
                wqk_r = wqk_d.rearrange("(k p) n -> p k n", p=128)
                weng = nc.gpsimd if cfg["dma2"] else nc.sync
                for k in range(KT):
                    nc.sync.dma_start(XT0[:, k, :], xt0_r[:, k, :])
                    weng.dma_start(Wqk[:, k, :], wqk_r[:, k, :])
                Bqk = consts.tile([128, MQK], F32)
                nc.sync.dma_start(Bqk, bqk_d[:])
                nc.sync.dma_start(Wv, wv_r)
                Mask = consts.tile([128, 128], BF16)
                nc.sync.dma_start(Mask, mask_d[:])
                Bv = consts.tile([128, C], F32)
                nc.sync.dma_start(Bv, bv_d[:])
            if cfg["tmav"]:
                Ident = consts.tile([128, 128], BF16)
                nc.sync.dma_start(Ident, ident_d[:])
            Wp = consts.tile([128, KT, C], BF16)
            nc.sync.dma_start(Wp, wp_d.rearrange("(k p) n -> p k n", p=128))
            Bp = consts.tile([128, C], F32)
            nc.sync.dma_start(Bp, bp_d[:])
            if cfg["tmav"] and cfg["maskp"]:
                nc.gpsimd.load_library(library_config.standard)
            elif not cfg["tmav"]:
                nc.gpsimd.load_library(library_config.attn)
            pending_g4 = []

            for b in range(NB):
                # ---- load xT for this batch ----
                if b == 0:
                    XT = XT0
                    if cfg["g1f8"]:
                        XT8 = XT8_0
                else:
                    if cfg["g1f8"]:
                        XT8 = xt8_pool.tile([128, KT, T], F8, tag="xt8")
                        nc.sync.dma_start(
                            XT8, xT8_d[b].rearrange("(k p) t -> p k t", p=128)
                        )
                    XT = xt_pool.tile([128, KT, T], BF16, tag="xt")
                    nc.sync.dma_start(XT, xT_d[b].rearrange("(k p) t -> p k t", p=128))

                # ---- GEMM1: qkT [2C, T], feature-major ----
                # emit m-tiles in (q-tile, k-tile) pairs so head h unblocks
                # after 2 m-tiles instead of after the whole q half
                QKT = qkt_pool.tile([128, MQK, T], BF16)

                def gemm1():
                    m_order = [
                        m for qt_ in range(MQK // 2) for m in (qt_, MQK // 2 + qt_)
                    ]
                    for j, m in enumerate(m_order):
                        qk_ps = ps_mm.tile([128, T], F32, tag="mm")
                        if cfg["g1f8"]:
                            # fp8 DoubleRow: k-tile pairs (2g, 2g+1) packed in
                            # the free dim; PSUM gets (SX8*SW8)*(x @ wqk)
                            wcol = j if cfg["wqk8chunk"] else m
                            for g in range(KT // 2):
                                nc.tensor.matmul(
                                    qk_ps,
                                    Wqk8[:, 2 * g : 2 * g + 2, 128 * wcol : 128 * (wcol + 1)],
                                    XT8[:, 2 * g : 2 * g + 2, :],
                                    start=(g == 0),
                                    stop=(g == KT // 2 - 1),
                                    perf_mode=DR,
                                )
                            if cfg["g1epi_dve"] and j % 2 == 1:
                                nc.vector.tensor_scalar(
                                    QKT[:, m, :], qk_ps, 1.0 / (SX8 * SW8),
                                    Bqk[:, m : m + 1],
                                    mybir.AluOpType.mult, mybir.AluOpType.add,
                                )
                            else:
                                nc.scalar.activation(
                                    QKT[:, m, :], qk_ps, AF.Identity,
                                    bias=Bqk[:, m : m + 1], scale=1.0 / (SX8 * SW8),
                                )
                        else:
                            for k in range(KT):
                                nc.tensor.matmul(
                                    qk_ps,
                                    Wqk[:, k, 128 * m : 128 * (m + 1)],
                                    XT[:, k, :],
                                    start=(k == 0),
                                    stop=(k == KT - 1),
                                )
                            nc.scalar.activation(
                                QKT[:, m, :], qk_ps, AF.Identity, bias=Bqk[:, m : m + 1]
                            )

                # ---- GEMM2: v_aug [T, H, D+1], token-major with ones column ----
                VA = va_pool.tile([128, TT, H, D + 1], BF16)

                def gemm2():
                    nc.vector.memset(VA[:, :, :, D : D + 1], 1.0)
                    for t in range(TT):
                        for n0, nw in ((0, 512), (512, 256)):
                            v_ps = ps_mm.tile([128, T], F32, tag="mm")
                            for k in range(KT):
                                nc.tensor.matmul(
                                    v_ps[:, :nw],
                                    XT[:, k, 128 * t : 128 * (t + 1)],
                                    Wv[:, k, n0 : n0 + nw],
                                    start=(k == 0),
                                    stop=(k == KT - 1),
                                )
                            nc.vector.tensor_tensor(
                                VA[:, t, n0 // D : (n0 + nw) // D, 0:D],
                                v_ps[:, :nw].rearrange("p (h d) -> p h d", d=D),
                                Bv[:, n0 : n0 + nw].rearrange("p (h d) -> p h d", d=D),
                                mybir.AluOpType.add,
                            )

                # batch 0: the v weights (2MB) arrive before Wqk (3.5MB), so
                # GEMM2 first gets the PE going ~2us earlier
                if b == 0 and cfg["g2first"]:
                    gemm2()
                    gemm1()
                else:
                    gemm1()
                    gemm2()

                # ---- attention per head ----
                OT = ot_pool.tile([128, KT, T], BF16)

                def head_slices(h):
                    qt = h // 2
                    qr = D * (h % 2)
                    return (
                        QKT[qr : qr + D, qt, :],
                        QKT[qr : qr + D, MQK // 2 + qt, :],
                    )

                def st_exp_av(h, i, o_ps):
                    if cfg["tmav"]:
                        return st_exp_av_tm(h, i, o_ps)
                    qT_h, kT_h = head_slices(h)
                    n = T - 128 * i
                    st_ps = ps_st.tile([128, T], F32, tag="st")
                    nc.tensor.matmul(
                        st_ps[:, :n],
                        kT_h[:, 128 * i : 128 * (i + 1)],
                        qT_h[:, 128 * i : T],
                        start=True,
                        stop=True,
                    )
                    PT = pt_pool.tile([128, T], BF16)
                    if cfg["split2"] and n >= 256:
                        chunks = [(0, n // 256 * 128), (n // 256 * 128, n)]
                    else:
                        chunks = [(0, n)]
                    for c0, c1 in chunks:
                        nc.scalar.activation(
                            PT[:, c0:c1], st_ps[:, c0:c1], AF.Exp, scale=SCALE
                        )
                        if c0 == 0:
                            nc.vector.tensor_tensor(
                                PT[:, 0:128], PT[:, 0:128], Mask, mybir.AluOpType.mult
                            )
                        nc.tensor.matmul(
                            o_ps[0 : D + 1, 128 * i + c0 : 128 * i + c1],
                            VA[:, i, h, :],
                            PT[:, c0:c1],
                            start=(i == 0 and c0 == 0),
                            stop=(i == TT - 1 and c1 == n),
                        )

                def st_exp_av_tm(h, i, o_ps):
                    # token-major AV: o_ps [128 q, TT, D+1]; for k-tile i,
                    # accumulate into q-tiles j >= i. One accumulation-group
                    # start per PSUM bank (i==0, j==0 zeroes the whole bank).
                    # With expmerge, i==2 and i==3 share one st tile/exp call
                    # (i==2 at cols 0:256, i==3 at cols 256:384).
                    qT_h, kT_h = head_slices(h)
                    merged = cfg["expmerge"] and i >= 2
                    if merged and i == 3:
                        st_ps, PT = st_exp_av_tm.m23[h]
                        c_base = 256
                    else:
                        n = T - 128 * i if not merged else 384
                        st_ps = ps_st.tile([128, T], F32, tag="st")
                        PT = pt_pool.tile([128, T], BF16)
                        c_base = 0
                    n_i = T - 128 * i
                    nc.tensor.matmul(
                        st_ps[:, c_base : c_base + n_i],
                        kT_h[:, 128 * i : 128 * (i + 1)],
                        qT_h[:, 128 * i : T],
                        start=True,
                        stop=True,
                    )
                    if merged and i == 2:
                        st_exp_av_tm.m23[h] = (st_ps, PT)
                        return  # exp + AV for i=2,3 issued when i==3 arrives
                    if merged:
                        # one exp over both i=2 (cols 0:256) and i=3 (256:384)
                        nc.scalar.activation(
                            PT[:, 0:384], st_ps[:, 0:384], AF.Exp, scale=SCALE
                        )
                        meng = nc.gpsimd if cfg["maskp"] else nc.vector
                        meng.tensor_tensor(
                            PT[:, 0:128], PT[:, 0:128], Mask, mybir.AluOpType.mult
                        )
                        meng.tensor_tensor(
                            PT[:, 256:384], PT[:, 256:384], Mask, mybir.AluOpType.mult
                        )
                        for ii, cb in ((2, 0), (3, 256)):
                            for j in range(ii, TT):
                                c = cb + 128 * (j - ii)
                                nc.tensor.matmul(
                                    o_ps[:, j, :],
                                    PT[:, c : c + 128],
                                    VA[:, ii, h, :],
                                    start=False,
                                    stop=(ii == TT - 1 and j == TT - 1),
                                    skip_group_check=True,
                                )
                        return
                    n = n_i
                    nc.scalar.activation(PT[:, :n], st_ps[:, :n], AF.Exp, scale=SCALE)
                    meng = nc.gpsimd if cfg["maskp"] else nc.vector
                    meng.tensor_tensor(
                        PT[:, 0:128], PT[:, 0:128], Mask, mybir.AluOpType.mult
                    )
                    for j in range(i, TT):
                        c = 128 * (j - i)
                        nc.tensor.matmul(
                            o_ps[:, j, :],
                            PT[:, c : c + 128],
                            VA[:, i, h, :],
                            start=(i == 0 and j == 0),
                            stop=(i == TT - 1 and j == TT - 1),
                            skip_group_check=True,
                        )
                st_exp_av_tm.m23 = {}

                def normalize_tm(h, o_ps, tp):
                    # per-partition normalize, then PE-transpose into tp
                    qr = D * (h % 2)
                    rinv = small_pool.tile([128, TT, 1], F32, tag="rinv_tm")
                    nc.vector.reciprocal(rinv, o_ps[:, :, D : D + 1])
                    oN = small_pool.tile([128, TT, D], BF16, tag="on_tm")
                    nc.vector.tensor_tensor(
                        oN,
                        o_ps[:, :, 0:D],
                        rinv.broadcast_to((128, TT, D)),
                        mybir.AluOpType.mult,
                    )
                    for j in range(TT):
                        nc.tensor.matmul(
                            tp[qr : qr + D, 128 * j : 128 * (j + 1)],
                            oN[:, j, :],
                            Ident,
                            is_transpose=True,
                            start=(j == 0),
                            stop=(j == TT - 1),
                            skip_group_check=True,
                        )

                def normalize(h, o_ps):
                    # normalize: oT_h = o[:D] / l, l = o row D
                    qt = h // 2
                    qr = D * (h % 2)
                    rinv = small_pool.tile([1, T], F32, tag="rinv")
                    if cfg["div"] == 0:
                        nc.vector.reciprocal(rinv, o_ps[D : D + 1, :])
                    elif cfg["div"] == 1:
                        nc.scalar.copy(rinv, o_ps[D : D + 1, :])
                    elif cfg["div"] == 2:
                        nc.vector.tensor_copy(rinv, o_ps[D : D + 1, :])
                    else:
                        nc.any.tensor_copy(rinv, o_ps[D : D + 1, :])
                    norm_op = (
                        mybir.AluOpType.mult if cfg["div"] == 0 else mybir.AluOpType.divide
                    )
                    rb = small_pool.tile([D, T], F32, tag="rb_sb")
                    nc.gpsimd.partition_broadcast(rb, rinv[:])
                    if cfg["ou"] == 0:
                        nc.vector.tensor_tensor(
                            OT[qr : qr + D, qt, :],
                            o_ps[0:D, :],
                            rb,
                            norm_op,
                        )
                    else:
                        oU = small_pool.tile([D, T], F32, tag="ou_sb")
                        if cfg["ou"] == 1:
                            nc.scalar.copy(oU, o_ps[0:D, :])
                        elif cfg["ou"] == 4:
                            nc.any.tensor_copy(oU, o_ps[0:D, :])
                        elif cfg["ou"] == 5 and h % 2 == 1:
                            nc.scalar.copy(oU, o_ps[0:D, :])
                        else:
                            nc.vector.tensor_copy(oU, o_ps[0:D, :])
                        eng = nc.gpsimd if cfg["norm_pool"] else nc.vector
                        eng.tensor_tensor(
                            OT[qr : qr + D, qt, :],
                            oU,
                            rb,
                            norm_op,
                        )

                if cfg["tmav"]:
                    for pair in range(H // 2):
                        hA, hB = 2 * pair, 2 * pair + 1
                        oA = ps_o.tile([128, TT, D + 1], F32, tag="o")
                        oB = ps_o.tile([128, TT, D + 1], F32, tag="o")
                        for i in range(TT):
                            st_exp_av_tm(hA, i, oA)
                            st_exp_av_tm(hB, i, oB)
                        # both heads transpose into one [128, T] tile (rows
                        # 0:64 / 64:128), copied as one DVE op into OT
                        tp = ps_tp.tile([128, T], BF16, tag="tp")
                        normalize_tm(hA, oA, tp)
                        normalize_tm(hB, oB, tp)
                        nc.vector.tensor_copy(OT[:, pair, :], tp)
                elif cfg["pairs"]:
                    # paired emission: the two heads of a QKT tile alternate, so
                    # their K=64 ST matmuls sit adjacently at row groups 0/64
                    # (concurrent on HW via tile_position row packing)
                    for pair in range(H // 2):
                        hA, hB = 2 * pair, 2 * pair + 1
                        oA = ps_o.tile([128, T], F32, tag="o")
                        oB = ps_o.tile([128, T], F32, tag="o")
                        for i in range(TT):
                            st_exp_av(hA, i, oA)
                            st_exp_av(hB, i, oB)
                        normalize(hA, oA)
                        normalize(hB, oB)
                else:
                    for h in range(H):
                        o_ps = ps_o.tile([128, T], F32, tag="o")
                        for i in range(TT):
                            st_exp_av(h, i, o_ps)
                        normalize(h, o_ps)

                # ---- GEMM4: y = o @ W_proj + b ----
                def gemm4(b=b, OT=OT):
                    last = b == NB - 1
                    for t in range(TT):
                        y_sb = y_pool.tile(
                            [128, C], BF16 if cfg["ybf16"] else F32, tag="ysb"
                        )
                        for ci, (n0, nw) in enumerate(((0, 512), (512, 256))):
                            # last batch: no next-batch GEMM1 needs the mm slots,
                            # so alternate pools for 4 accumulation groups in flight
                            if cfg["y"] == 0 or (
                                cfg["tail4"] and last and (2 * t + ci) % 2 == 1
                            ):
                                y_ps = ps_mm.tile([128, T], F32, tag="mm")
                            else:
                                y_ps = ps_y.tile([128, T], F32, tag="y")
                            for k in range(KT):
                                nc.tensor.matmul(
                                    y_ps[:, :nw],
                                    OT[:, k, 128 * t : 128 * (t + 1)],
                                    Wp[:, k, n0 : n0 + nw],
                                    start=(k == 0),
                                    stop=(k == KT - 1),
                                )
                            nc.vector.tensor_tensor(
                                y_sb[:, n0 : n0 + nw],
                                y_ps[:, :nw],
                                Bp[:, n0 : n0 + nw],
                                mybir.AluOpType.add,
                            )
                        if cfg["ysplit"] and last:
                            for n0, nw in ((0, 512), (512, 256)):
                                nc.sync.dma_start(
                                    y_d[b, 128 * t : 128 * (t + 1), n0 : n0 + nw],
                                    y_sb[:, n0 : n0 + nw],
                                )
                        else:
                            nc.sync.dma_start(y_d[b, 128 * t : 128 * (t + 1), :], y_sb)

                if cfg["g4defer"]:
                    pending_g4.append(gemm4)
                    if b >= 1:
                        pending_g4.pop(0)()
                else:
                    gemm4()

            if cfg["g4defer"]:
                for fn in pending_g4:
                    fn()

    return nc


_NC_CACHE = None


def _get_nc():
    global _NC_CACHE
    if _NC_CACHE is None:
        nc = build_bass()
        nc.finalize()
        _NC_CACHE = nc
    return _NC_CACHE


def make_in_maps(x, w_qkv, b_qkv, b_proj, w_proj, cfg=None):
    cfg = {**DEFAULT_CFG, **(cfg or {})}
    x = np.asarray(x, np.float32)
    w_qkv = np.asarray(w_qkv, np.float32)
    b_qkv = np.asarray(b_qkv, np.float32)
    w_proj = np.asarray(w_proj, np.float32)
    b_proj = np.asarray(b_proj, np.float32)
    wv = np.ascontiguousarray(w_qkv[:, 2 * C :]).astype(BF16NP)
    wp = np.asarray(w_proj).astype(BF16NP)
    bqk = np.ascontiguousarray(
        np.asarray(b_qkv[: 2 * C], np.float32).reshape(MQK, 128).T
    )
    bv = np.broadcast_to(np.asarray(b_qkv[2 * C :], np.float32), (128, C)).copy()
    bp = np.broadcast_to(np.asarray(b_proj, np.float32), (128, C)).copy()
    kk, qq = np.meshgrid(np.arange(128), np.arange(128), indexing="ij")
    mask = (kk <= qq).astype(BF16NP)

    shared = {"wv": wv, "wp": wp, "bqk": bqk, "bv": bv, "bp": bp, "mask": mask}
    if cfg["tmav"]:
        shared["ident"] = np.eye(128, dtype=np.float32).astype(BF16NP)
    if cfg["g1f8"]:
        wqk8 = np.ascontiguousarray(w_qkv[:, : 2 * C] * SW8).astype(F8NP)
        if cfg["wqk8chunk"]:
            m_order = [m for qt in range(MQK // 2) for m in (qt, MQK // 2 + qt)]
            perm = np.concatenate([np.arange(128 * m, 128 * (m + 1)) for m in m_order])
            wqk8 = np.ascontiguousarray(wqk8[:, perm])
        shared["wqk8"] = wqk8
    else:
        shared["wqk"] = np.ascontiguousarray(w_qkv[:, : 2 * C]).astype(BF16NP)

    in_maps = []
    for c in range(NCORES):
        xc = np.asarray(x[c * NB : (c + 1) * NB], np.float32)
        xcT = np.ascontiguousarray(xc.transpose(0, 2, 1))
        m = {"xT": xcT.astype(BF16NP), **shared}
        if cfg["g1f8"]:
            m["xT8"] = (xcT * SX8).astype(F8NP)
        in_maps.append(m)
    return in_maps


def kernel(x, w_qkv, b_qkv, w_proj, b_proj, _trace=False, _tmpdir=None):
    x = np.asarray(x)
    in_maps = make_in_maps(x, w_qkv, b_qkv, b_proj, w_proj)
    nc = _get_nc()
    res = run_bass_kernel_spmd(
        nc, in_maps, list(range(NCORES)), trace=_trace, tmpdir=_tmpdir
    )
    out = np.concatenate([np.asarray(r["y"], np.float32) for r in res.results], axis=0)
    if _trace:
        kernel.last_exec_time_ns = res.exec_time_ns
        kernel.last_results = res
    return out.reshape(B, T, C)


if __name__ == "__main__":
    rng = np.random.default_rng(0)
    x = rng.standard_normal((B, T, C), dtype=np.float32)
    w_qkv = (rng.standard_normal((C, 3 * C), dtype=np.float32) * 0.02).astype(np.float32)
    b_qkv = np.zeros((3 * C,), np.float32)
    w_proj = (rng.standard_normal((C, C), dtype=np.float32) * 0.02).astype(np.float32)
    b_proj = np.zeros((C,), np.float32)
    y = kernel(x, w_qkv=w_qkv, b_qkv=b_qkv, w_proj=w_proj, b_proj=b_proj)
    print(y.shape, y.dtype)

